# revision 1
# baseline (speedup 1.0000x reference)
"""Trainium2 Bass kernel for nn_Attention_80384607912675.

Multi-head attention (B=2, S=2048, D=1024, H=16, HD=64), fp32 reference.

Sharding (8 cores): data-parallel over batch (2) x tensor-parallel over heads
(4 head groups of 4 heads).  Core c handles batch c//4, heads [4*(c%4), 4*(c%4)+4).
wq/wk/wv split column-wise, wo split row-wise; the wo partial sums (and the
bias bo) are reduced on the host.

Per-core kernel (all matmuls bf16 with fp32 PSUM accumulation):
  QT/KT = (x @ wq/k + b)^T  stored head-major [256, 2048]
  V_aug = [x @ wv + bv | 1] stored natural    [2048, 4*(64+1)]  (ones column
                            per head folds the softmax row-sum into PV)
  per head pair hp (A/B), q-window qw (512 wide), kp-chunk c:
    S^T[kp, q]   = K_h^T (x) Q_h   (K=64; A,B packed side-by-side in one
                                    PSUM tile, row-tiled in the PE)
    P^T          = exp(S^T / 8)    (one ScalarE instr per A|B pair, ->bf16)
    [O^T; rowsum] += V_aug^T (x) P^T   (M=65, accumulated over c)
  O^T normalized by broadcast(1/rowsum) (PE K=1 broadcast + DVE multiply)
  out_partial = O_norm^T.T @ wo_c -> [2048, 1024] bf16 (heads 0-1 staged
  under the second attention pair, heads 2-3 added in the tail)

The exp (ScalarE) stream is the bottleneck (~134us busy); all PE-side work
(projections, normalization, output projection) is interleaved into its slack
via a static filler schedule, and the lead-in QK projections pipeline against
the streamed xT DMA chunks.
"""

import numpy as np

B, S, D, H = 2, 2048, 1024, 16
HD = D // H          # 64
HPC = 4              # heads per core
DHC = HPC * HD       # 256 head dims per core
KC = D // 128        # 8 contraction chunks
SB = S // 128        # 16 s blocks / kp chunks
VP = HPC * (HD + 1)  # 260: V storage pitch per s-chunk (ones col per head)
NC = 8               # cores
NQW = 4              # 512-wide q windows per head pair

_nc_cache = {}


def _build_bass(debug=False, with_bias=False):
    import concourse.mybir as mybir
    import concourse.tile as tile
    from concourse import bacc

    BF = mybir.dt.bfloat16
    F32 = mybir.dt.float32
    EXP = mybir.ActivationFunctionType.Exp

    nc = bacc.Bacc("TRN2")

    xT_d = nc.dram_tensor("xT", [D, S], BF, kind="ExternalInput")
    wq_d = nc.dram_tensor("wq_c", [D, DHC], BF, kind="ExternalInput")
    wk_d = nc.dram_tensor("wk_c", [D, DHC], BF, kind="ExternalInput")
    wv_d = nc.dram_tensor("wv_c", [D, DHC], BF, kind="ExternalInput")
    wo_d = nc.dram_tensor("wo_c", [DHC, D], BF, kind="ExternalInput")
    bias_d = nc.dram_tensor("bias3", [1, 3 * DHC], BF, kind="ExternalInput")
    out1_d = nc.dram_tensor("out1", [S, D], BF, kind="ExternalOutput")
    out2_d = nc.dram_tensor("out2", [S, D], BF, kind="ExternalOutput")
    if debug:
        dbg = {
            "qt": nc.dram_tensor("dbg_qt", [128, 2 * S], BF, kind="ExternalOutput"),
            "kt": nc.dram_tensor("dbg_kt", [128, 2 * S], BF, kind="ExternalOutput"),
            "v": nc.dram_tensor("dbg_v", [128, SB * VP], BF, kind="ExternalOutput"),
            "oun": nc.dram_tensor(
                "dbg_oun", [HD + 1, HPC * S], F32, kind="ExternalOutput"
            ),
            "onm": nc.dram_tensor("dbg_onm", [HD, HPC * S], BF, kind="ExternalOutput"),
            "onm2": nc.dram_tensor("dbg_onm2", [128, 2 * S], BF, kind="ExternalOutput"),
            "wo2": nc.dram_tensor("dbg_wo2", [128, 2 * D], BF, kind="ExternalOutput"),
            "ost": nc.dram_tensor("dbg_ost", [128, SB * D], BF, kind="ExternalOutput"),
        }

    with tile.TileContext(nc) as tc:
        with (
            tc.tile_pool(name="persist", bufs=1) as pp,
            tc.tile_pool(name="sc", bufs=2, space="PSUM") as scp,
            tc.tile_pool(name="oacc", bufs=1, space="PSUM") as opp,
            tc.tile_pool(name="pj", bufs=2, space="PSUM") as pjp,
            tc.tile_pool(name="pt", bufs=6) as ptp,
            tc.tile_pool(name="rc", bufs=2) as rcp,
            tc.tile_pool(name="bb", bufs=4) as bbp,
            tc.tile_pool(name="osb", bufs=6) as oup,
        ):
            xT_sb = pp.tile([128, KC * S], BF, tag="xT", name="xT_sb")
            wq_sb = pp.tile([128, KC * DHC], BF, tag="wq", name="wq_sb")
            wk_sb = pp.tile([128, KC * DHC], BF, tag="wk", name="wk_sb")
            wv_sb = pp.tile([128, KC * DHC], BF, tag="wv", name="wv_sb")
            wo_sb = pp.tile([128, 2 * D], BF, tag="wo", name="wo_sb")
            qt_sb = pp.tile([128, 2 * S], BF, tag="qt", name="qt_sb")
            kt_sb = pp.tile([128, 2 * S], BF, tag="kt", name="kt_sb")
            v_sb = pp.tile([128, SB * VP], BF, tag="v", name="v_sb")
            vt_sb = pp.tile([128, 2 * S], BF, tag="vt", name="vt_sb")
            ident = pp.tile([128, 128], BF, tag="ident", name="ident")
            oun_sb = pp.tile([HD + 1, HPC * S], F32, tag="oun", name="oun_sb")
            onm_sb = pp.tile([HD, HPC * S], BF, tag="onm", name="onm_sb")
            onm2_sb = pp.tile([128, 2 * S], BF, tag="onm2", name="onm2_sb")
            bias_sb = pp.tile([1, 3 * DHC], BF, tag="bias", name="bias_sb")
            ones16 = pp.tile([1, 512], BF, tag="ones16", name="ones16")

            # input DMAs: small weights first; xT streamed in 8 chunks that the
            # lead-in projections consume as they land; wo last (needed late)
            def load_w(w_sb, w_d):
                nc.sync.dma_start(
                    w_sb[:, :].rearrange("p (k d) -> p k d", d=DHC),
                    w_d[:, :].rearrange("(k p) d -> p k d", p=128),
                )

            def load_xt(k):
                nc.sync.dma_start(
                    xT_sb[:, k * S:(k + 1) * S], xT_d[k * 128:(k + 1) * 128, :]
                )

            load_w(wq_sb, wq_d)
            for k in range(4):
                load_xt(k)
            load_w(wk_sb, wk_d)
            for k in range(4, KC):
                load_xt(k)
            load_w(wv_sb, wv_d)
            nc.sync.dma_start(bias_sb[:, :], bias_d[:, :])
            nc.sync.dma_start(
                wo_sb[:, :].rearrange("r (p d) -> r p d", d=D),
                wo_d[:, :].rearrange("(p r) d -> r p d", r=128),
            )
            nc.vector.memset(ones16[:, :], 1.0)
            # ones columns of V_aug: preset everything to 1, V overwrites below
            nc.gpsimd.memset(v_sb[:, :], 1.0)
            from concourse.masks import make_identity
            make_identity(nc, ident[:, :])

            bq = bias_sb[0:1, 0:DHC]
            bk = bias_sb[0:1, DHC:2 * DHC]
            bv = bias_sb[0:1, 2 * DHC:3 * DHC]

            def qk_mm(ps, w_sb, p, nt, k):
                nc.tensor.matmul(
                    ps[:, :],
                    lhsT=w_sb[:, k * DHC + p * 128: k * DHC + (p + 1) * 128],
                    rhs=xT_sb[:, k * S + nt * 512: k * S + (nt + 1) * 512],
                    start=(k == 0),
                    stop=(k == KC - 1 and not with_bias),
                )

            def qk_fin(ps, dst, bias, p, nt, on_act=False):
                if with_bias:
                    nc.tensor.matmul(
                        ps[:, :],
                        lhsT=bias[:, p * 128:(p + 1) * 128],
                        rhs=ones16[0:1, :],
                        start=False,
                        stop=True,
                    )
                dslice = dst[:, p * S + nt * 512: p * S + (nt + 1) * 512]
                if on_act:
                    nc.scalar.copy(dslice, ps[:, :])
                else:
                    nc.vector.tensor_copy(dslice, ps[:, :])

            _qk_pending = {}

            def proj_qk_a(dst, w_sb, bias, p, nt):
                ps = pjp.tile([128, 512], F32, tag="pj", name=f"qk_{p}_{nt}")
                for k in range(KC // 2):
                    qk_mm(ps, w_sb, p, nt, k)
                _qk_pending[(p, nt, dst.tensor.name)] = ps

            def proj_qk_b(dst, w_sb, bias, p, nt):
                ps = _qk_pending.pop((p, nt, dst.tensor.name))
                for k in range(KC // 2, KC):
                    qk_mm(ps, w_sb, p, nt, k)
                qk_fin(ps, dst, bias, p, nt)

            def proj_vt(db, nt, pool=None, tag="pj"):
                """V^T[d-block db, s-window nt]: (wv^T x^T + bv) -> vt_sb bf16."""
                ps = (pool or pjp).tile([128, 512], F32, tag=tag, name=f"vt_{db}_{nt}")
                for k in range(KC):
                    nc.tensor.matmul(
                        ps[:, :],
                        lhsT=wv_sb[:, k * DHC + db * 128: k * DHC + (db + 1) * 128],
                        rhs=xT_sb[:, k * S + nt * 512: k * S + (nt + 1) * 512],
                        start=(k == 0),
                        stop=(k == KC - 1 and not with_bias),
                    )
                if with_bias:
                    nc.tensor.matmul(
                        ps[:, :],
                        lhsT=bv[:, db * 128:(db + 1) * 128],
                        rhs=ones16[0:1, :],
                        start=False,
                        stop=True,
                    )
                nc.vector.tensor_copy(
                    vt_sb[:, db * S + nt * 512: db * S + (nt + 1) * 512], ps[:, :]
                )

            def v_tp(sb, db):
                """Transpose V^T block (d-block db, s-chunk sb) into v_sb."""
                tp = pjp.tile([128, 128], BF, tag="pj", name=f"tp_{sb}_{db}")
                nc.tensor.transpose(
                    tp[:, :], vt_sb[:, db * S + sb * 128: db * S + (sb + 1) * 128],
                    ident[:, :],
                )
                dst3 = v_sb[
                    :, sb * VP + 2 * db * (HD + 1): sb * VP + (2 * db + 2) * (HD + 1)
                ].rearrange("p (h e) -> p h e", e=HD + 1)[:, :, 0:HD]
                nc.vector.tensor_copy(dst3, tp[:, :])

            def outproj_piece(sb, n, pair, out_dram, on_act=False):
                """Half s-block head-pair partial -> bf16 -> DMA."""
                ot = oup.tile([128, 512], BF, tag="osb", name=f"ot{pair}_{sb}_{n}")
                po = pjp.tile([128, 512], F32, tag="pj", name=f"po{pair}_{sb}_{n}")
                nc.tensor.matmul(
                    po[:, :],
                    lhsT=onm2_sb[:, pair * S + sb * 128: pair * S + (sb + 1) * 128],
                    rhs=wo_sb[:, pair * D + n * 512: pair * D + (n + 1) * 512],
                    start=True,
                    stop=True,
                )
                if on_act:
                    nc.scalar.copy(ot[:, :], po[:, :])
                else:
                    nc.vector.tensor_copy(ot[:, :], po[:, :])
                nc.sync.dma_start(
                    out_dram[sb * 128:(sb + 1) * 128, n * 512:(n + 1) * 512], ot[:, :]
                )

            def outproj1(sb, n):
                outproj_piece(sb, n, 0, out1_d)

            def outproj2(sb, on_act=False):
                for n in range(2):
                    outproj_piece(sb, n, 1, out2_d, on_act=on_act)

            # ---- lead-in: QT/KT p0 all nt, k-major across 4 psum slots so the
            # matmuls pipeline against the arriving xT chunks
            lead = [
                (qt_sb, wq_sb, bq, 0, 0, pjp, "pj"),
                (kt_sb, wk_sb, bk, 0, 0, pjp, "pj"),
                (kt_sb, wk_sb, bk, 0, 1, scp, "sc"),
                (qt_sb, wq_sb, bq, 0, 1, scp, "sc"),
                (None, wv_sb, bv, 0, 0, opp, "oacc"),  # V^T(0,0)
            ]
            lead_ps = [
                pool.tile([128, 512], F32, tag=tag, name=f"lead_{nt}_{tag}")
                for dst, w_sb, bias, p, nt, pool, tag in lead
            ]
            for k in range(KC):
                for (dst, w_sb, bias, p, nt, pool, tag), ps in zip(lead, lead_ps):
                    if dst is None:
                        nc.tensor.matmul(
                            ps[:, :],
                            lhsT=wv_sb[:, k * DHC + p * 128: k * DHC + (p + 1) * 128],
                            rhs=xT_sb[:, k * S + nt * 512: k * S + (nt + 1) * 512],
                            start=(k == 0),
                            stop=(k == KC - 1 and not with_bias),
                        )
                    else:
                        qk_mm(ps, w_sb, p, nt, k)
            for (dst, w_sb, bias, p, nt, pool, tag), ps in zip(lead, lead_ps):
                if dst is None:
                    if with_bias:
                        nc.tensor.matmul(
                            ps[:, :],
                            lhsT=bv[:, p * 128:(p + 1) * 128],
                            rhs=ones16[0:1, :],
                            start=False,
                            stop=True,
                        )
                    nc.scalar.copy(
                        vt_sb[:, p * S + nt * 512: p * S + (nt + 1) * 512], ps[:, :]
                    )
                else:
                    qk_fin(ps, dst, bias, p, nt, on_act=True)

            # ---- filler schedule: (hp, qw, c) -> deferred work emitted inside
            # the ACT-bound attention loop
            fillers = {}

            def add(hp, qw, c, fn):
                fillers.setdefault((hp, qw, c), []).append(fn)

            def add_qk(hp, qw, c, dst, w_sb, bias, p, nt):
                add(hp, qw, c, lambda: proj_qk_a(dst, w_sb, bias, p, nt))
                add(hp, qw, c + 1, lambda: proj_qk_b(dst, w_sb, bias, p, nt))

            for c in range(SB):  # heads 0,1 transposes JIT (vt block c//4 ready)
                add(0, 0, c, lambda c=c: v_tp(c, 0))
            add(0, 0, 1, lambda: proj_vt(0, 1))
            add(0, 0, 5, lambda: proj_vt(0, 2))
            add(0, 0, 9, lambda: proj_vt(0, 3))
            add_qk(0, 0, 2, kt_sb, wk_sb, bk, 0, 2)
            add_qk(0, 0, 6, kt_sb, wk_sb, bk, 0, 3)
            for nt in range(4):  # V^T heads 2,3 (needed from hp1)
                add(0, 1, 4 * nt, lambda nt=nt: proj_vt(1, nt))
            for i in range(8):
                add(0, 1, 2 * i + 1, lambda sb=i: v_tp(sb, 1))
                add(0, 2, 2 * i, lambda sb=i + 8: v_tp(sb, 1))
            add_qk(0, 1, 1, qt_sb, wq_sb, bq, 0, 2)
            add_qk(0, 1, 9, qt_sb, wq_sb, bq, 0, 3)
            add_qk(0, 2, 1, kt_sb, wk_sb, bk, 1, 0)
            add_qk(0, 2, 9, kt_sb, wk_sb, bk, 1, 1)
            add_qk(0, 3, 0, kt_sb, wk_sb, bk, 1, 2)
            add_qk(0, 3, 4, kt_sb, wk_sb, bk, 1, 3)
            add_qk(0, 3, 8, qt_sb, wq_sb, bq, 1, 0)
            add_qk(0, 3, 12, qt_sb, wq_sb, bq, 1, 1)
            add_qk(1, 0, 2, qt_sb, wq_sb, bq, 1, 2)
            add_qk(1, 0, 6, qt_sb, wq_sb, bq, 1, 3)
            for i in range(32):  # outproj stage 1 spread over hp1 qw0/qw1
                sb, n = divmod(i, 2)
                add(1, i // 16, i % 16, lambda sb=sb, n=n: outproj1(sb, n))
            for i in range(8):  # outproj stage 2 for sb 0..7 under hp1 qw2
                add(1, 2, 1 + 2 * (i % 8), lambda sb=i: outproj2(sb))
            for i in range(8, 12):  # sb 8..11 under hp1 qw3 (after its drains)
                add(1, 3, 4 + 2 * (i - 8), lambda sb=i: outproj2(sb))

            def drain_window(hp, qw, oacc, part):
                """Deferred per-window drain: park+recip / bcast+norm+relocate."""
                hA = 2 * hp
                oun4 = oun_sb[:, :].rearrange("p (h s) -> p h s", h=HPC)
                if part == 0:
                    nc.vector.tensor_copy(
                        oun4[0:HD + 1, hA:hA + 2, qw * 512:(qw + 1) * 512],
                        oacc[:, :],
                    )
                    return
                rs0 = rcp.tile([1, 1024], F32, tag="rs0", name=f"rs0_{hp}{qw}")
                nc.sync.dma_start(
                    rs0[0:1, :],
                    oun4[HD:HD + 1, hA:hA + 2, qw * 512:(qw + 1) * 512],
                )
                rc = rcp.tile([1, 1024], F32, tag="rc", name=f"rc_{hp}{qw}")
                nc.vector.reciprocal_approx_fast(out=rc[0:1, :], in_=rs0[0:1, :])
                for i in range(2):
                    h = 2 * hp + i
                    pb = bbp.tile([HD, 512], F32, tag="bb", name=f"bb_{hp}{qw}{i}")
                    nc.gpsimd.partition_broadcast(
                        pb[:, :], rc[0:1, i * 512:(i + 1) * 512]
                    )
                    qcol = h * S + qw * 512
                    nc.vector.tensor_mul(
                        onm_sb[0:HD, qcol:qcol + 512],
                        oun_sb[0:HD, qcol:qcol + 512],
                        pb[:, :],
                    )
                    nc.sync.dma_start(
                        onm2_sb[64 * i:64 * (i + 1), hp * S + qw * 512:
                                hp * S + (qw + 1) * 512],
                        onm_sb[0:HD, qcol:qcol + 512],
                    )

            # ---- attention
            pending_drain = []
            for hp in range(2):
                for qw in range(NQW):
                    oacc = opp.tile(
                        [HD + 1, 1024], F32, tag="oacc", name=f"o_{hp}_{qw}"
                    )
                    prev = None

                    def emit_pv(pt_t, c, oacc=oacc, hp=hp):
                        for i in range(2):
                            nc.tensor.matmul(
                                oacc[:, i * 512:(i + 1) * 512],
                                lhsT=v_sb[
                                    :, c * VP + (HD + 1) * (2 * hp + i):
                                    c * VP + (HD + 1) * (2 * hp + i + 1)
                                ],
                                rhs=pt_t[:, 512 * i:512 * (i + 1)],
                                start=(c == 0),
                                stop=(c == SB - 1),
                            )

                    for c in range(SB):
                        sc = scp.tile(
                            [128, 1024], F32, tag="sc", name=f"sc_{hp}{qw}{c}"
                        )
                        for i in range(2):  # head A | head B packed
                            nc.tensor.matmul(
                                sc[:, 512 * i:512 * (i + 1)],
                                lhsT=kt_sb[
                                    64 * i:64 * (i + 1),
                                    hp * S + c * 128: hp * S + (c + 1) * 128,
                                ],
                                rhs=qt_sb[
                                    64 * i:64 * (i + 1),
                                    hp * S + qw * 512: hp * S + (qw + 1) * 512,
                                ],
                                start=True,
                                stop=True,
                            )
                        pt_t = ptp.tile(
                            [128, 1024], BF, tag="pt", name=f"pt_{hp}{qw}{c}"
                        )
                        nc.scalar.activation(pt_t[:, :], sc[:, :], EXP, scale=0.125)
                        if pending_drain and c == 0:
                            drain_window(*pending_drain[0], 0)
                        elif pending_drain and c == 2:
                            drain_window(*pending_drain.pop(0), 1)
                        for fn in fillers.get((hp, qw, c), ()):
                            fn()
                        if prev is not None:
                            emit_pv(prev, c - 1)
                        prev = pt_t
                    emit_pv(prev, SB - 1)
                    pending_drain.append((hp, qw, oacc))

            while pending_drain:
                hp, qw, oacc = pending_drain.pop(0)
                drain_window(hp, qw, oacc, 0)
                drain_window(hp, qw, oacc, 1)
            for sb in range(12, SB):
                outproj2(sb, on_act=True)

            if debug:
                nc.sync.dma_start(dbg["qt"][:, :], qt_sb[:, :])
                nc.sync.dma_start(dbg["kt"][:, :], kt_sb[:, :])
                nc.sync.dma_start(dbg["v"][:, :], v_sb[:, :])
                nc.sync.dma_start(dbg["oun"][:, :], oun_sb[:, :])
                nc.sync.dma_start(dbg["onm"][:, :], onm_sb[:, :])
                nc.sync.dma_start(dbg["onm2"][:, :], onm2_sb[:, :])
                nc.sync.dma_start(dbg["wo2"][:, :], wo_sb[:, :])
                nc.sync.dma_start(dbg["ost"][:, :], ost_sb[:, :])

    nc.compile()
    return nc


def _get_nc(with_bias=False):
    if with_bias not in _nc_cache:
        _nc_cache[with_bias] = _build_bass(with_bias=with_bias)
    return _nc_cache[with_bias]


def _prepare_in_maps(x, wq, bq, wk, bk, wv, bv, wo):
    import ml_dtypes

    bf16 = ml_dtypes.bfloat16
    x = np.asarray(x, np.float32)
    wq, bq = np.asarray(wq, np.float32), np.asarray(bq, np.float32)
    wk, bk = np.asarray(wk, np.float32), np.asarray(bk, np.float32)
    wv, bv = np.asarray(wv, np.float32), np.asarray(bv, np.float32)
    wo = np.asarray(wo, np.float32)

    xT = [np.ascontiguousarray(x[b].T).astype(bf16) for b in range(B)]
    in_maps = []
    for c in range(NC):
        b, j = divmod(c, HPC)
        cs = slice(DHC * j, DHC * (j + 1))
        bias3 = np.concatenate([bq[cs], bk[cs], bv[cs]]).reshape(1, 3 * DHC).astype(bf16)
        in_maps.append(
            {
                "xT": xT[b],
                "wq_c": np.ascontiguousarray(wq[:, cs]).astype(bf16),
                "wk_c": np.ascontiguousarray(wk[:, cs]).astype(bf16),
                "wv_c": np.ascontiguousarray(wv[:, cs]).astype(bf16),
                "wo_c": np.ascontiguousarray(wo[cs, :]).astype(bf16),
                "bias3": np.ascontiguousarray(bias3),
            }
        )
    return in_maps


def _gather(parts, bo):
    bo = np.asarray(bo, np.float32)
    out = np.empty((B, S, D), np.float32)
    for b in range(B):
        acc = np.asarray(parts[HPC * b], np.float32)
        for j in range(1, HPC):
            acc = acc + np.asarray(parts[HPC * b + j], np.float32)
        out[b] = acc + bo
    return out


def kernel(x, wq, bq, wk, bk, wv, bv, wo, bo):
    from concourse import bass_utils

    in_maps = _prepare_in_maps(x, wq, bq, wk, bk, wv, bv, wo)
    with_bias = bool(
        np.any(np.asarray(bq)) or np.any(np.asarray(bk)) or np.any(np.asarray(bv))
    )
    res = bass_utils.run_bass_kernel_spmd(
        nc=_get_nc(with_bias), in_maps=in_maps, core_ids=list(range(NC))
    )
    parts = [
        np.asarray(r["out1"], np.float32) + np.asarray(r["out2"], np.float32)
        for r in res.results
    ]
    return _gather(parts, bo)



# revision 72
# speedup vs baseline: 1.1328x; 1.1328x over previous
"""Trainium2 Bass kernel for nn_Attention_80384607912675.

Multi-head attention (B=2, S=2048, D=1024, H=16, HD=64), fp32 reference.

Sharding (8 cores): data-parallel over batch (2) x tensor-parallel over heads
(4 head groups of 4 heads).  Core c handles batch c//4, heads [4*(c%4), ...).
wq/wk/wv split column-wise, wo split row-wise; the 4 per-batch partials are
summed on the host (+bo).

Key structure (timings per the TRN2 cost model, matmul cost = N_free x rate):
  - QKV projections in fp8e4 DoubleRow (2 k-tiles/instr, 0.5 cyc/row) with an
    exact-to-~0.15% hi/lo split of x and w (split on host; 3 of 4 product
    terms kept).
  - Scores S^T = K^T (x) Q via a 256-deep augmented contraction
    [q_hi;q_lo]x2 * [[k_hi;k_hi],[k_lo;k_lo]] in ONE fp8 DoubleRow instr per
    (head, kp-chunk): exact product reconstruction in half the bf16 PE time.
    The aug layouts are built by partition-remap DMAs from the drain outputs.
  - exp on ScalarE ([128,1024] tiles from PSUM); a static subset of tiles is
    offloaded to DVE via a Schraudolph exp2 bit-trick (+Pool for the bf16
    convert) to rebalance engine load.
  - PV flipped: O[q, d] = sum_c P^T-chunk.T @ V_aug-chunk, N=65 per matmul
    (instead of 512): half the PE time.  V_aug's ones column gives the
    softmax rowsum in column 64 of each oacc sub-tile.
  - O normalized per (head, q-block) via tensor_scalar with per-partition
    reciprocal, PE-transposed to O^T for the wo matmul.
  - Output projection accumulates both head-pairs in PSUM (single out DRAM
    tensor per core).
  - PV lags QK/exp by 2 chunks so PE never waits on ScalarE in steady state;
    projections/outproj/transposes are spread as static fillers.
"""

import numpy as np

B, S, D, H = 2, 2048, 1024, 16
HD = D // H          # 64
HPC = 4              # heads per core
DHC = HPC * HD       # 256 head dims per core
KC = D // 128        # 8 contraction chunks of 128
NCH = S // 128       # 16 kp chunks / s blocks / q blocks
VP = HPC * (HD + 1)  # 260: V_aug pitch per s-chunk
NC = 8               # cores
NQW = 4              # 512-wide q windows per head pair
NW = 2 * NQW         # 8 windows

# oacc sub-tile col offsets (f32) for j = qb*2 + head_i; j=7 starts at bank 1
OFFS = [0, 65, 130, 195, 260, 325, 390, 512]

# (window, c) exp tiles computed on DVE+Pool instead of ScalarE; placed in
# windows with DVE slack, away from the slot-0/1 norm drains, so step-1 runs
# promptly and the sc slot frees before the hoisted QK two chunks later
SCHR = set()  # disabled: Schraudolph bias pushed rel err past the gate
# pair-0 windows (0..3) run bf16 QK: no fp8 aug prep deadlines on the lead /
# early windows; pair-1 windows (4..7) use the fp8 DoubleRow aug path whose
# prep has three windows of runway.
_LOG2E = 1.4426950408889634
# wq/wk/wv are pre-scaled by WSC on the host so their values (and the hi/lo
# fp8 split residuals) sit in e4m3's normal range; scores then carry a
# WSC^2 factor that is absorbed into the exp scale, and V's WSC into wo.
WSC = 32.0
EXPSC = 0.125 / (WSC * WSC)
SCHR_A = EXPSC * _LOG2E * (1 << 23)
SCHR_B = float(127 * (1 << 23)) - 366000.0

WNAMES = ("qh", "ql", "kh", "kl", "vh", "vl")

_nc_cache = {}


def _build_bass(with_bias=False, debug=False):
    import concourse.mybir as mybir
    import concourse.tile as tile
    from concourse import bacc

    BF = mybir.dt.bfloat16
    F32 = mybir.dt.float32
    FP8 = mybir.dt.float8e4
    I32 = mybir.dt.int32
    EXP = mybir.ActivationFunctionType.Exp
    DR = mybir.MatmulPerfMode.DoubleRow
    MUL = mybir.AluOpType.mult
    ADD = mybir.AluOpType.add
    SUB = mybir.AluOpType.subtract

    nc = bacc.Bacc("TRN2")

    xh_d = nc.dram_tensor("xh", [D, S], FP8, kind="ExternalInput")
    xl_d = nc.dram_tensor("xl", [D, S], FP8, kind="ExternalInput")
    w8_d = nc.dram_tensor("w8", [128, 6 * KC * DHC], FP8, kind="ExternalInput")
    wo_d = nc.dram_tensor("wo_c", [DHC, D], BF, kind="ExternalInput")
    if with_bias:
        bias_d = nc.dram_tensor("bias3", [1, 3 * DHC], BF, kind="ExternalInput")
    out_d = nc.dram_tensor("out", [S, D], BF, kind="ExternalOutput")
    if debug:
        dbg = {
            "v": nc.dram_tensor("dbg_v", [128, NCH * VP], BF,
                                kind="ExternalOutput"),
            "ktbf": nc.dram_tensor("dbg_ktbf", [128, 2048], BF,
                                   kind="ExternalOutput"),
            "qtbf": nc.dram_tensor("dbg_qtbf", [128, NQW * 512], BF,
                                   kind="ExternalOutput"),
            "qaug": nc.dram_tensor("dbg_qaug", [128, HPC * S],
                                   mybir.dt.float8e4, kind="ExternalOutput"),
            "kaug": nc.dram_tensor("dbg_kaug", [128, 2 * HPC * S],
                                   mybir.dt.float8e4, kind="ExternalOutput"),
            "onT": nc.dram_tensor("dbg_onT", [128, 2 * S], BF,
                                  kind="ExternalOutput"),
            "oacc": nc.dram_tensor("dbg_oacc", [128, 1024], mybir.dt.float32,
                                   kind="ExternalOutput"),
            "r": nc.dram_tensor("dbg_r", [128, 8], mybir.dt.float32,
                                kind="ExternalOutput"),
            "onm": nc.dram_tensor("dbg_onm", [128, 4 * 128], BF,
                                  kind="ExternalOutput"),
        }

    with tile.TileContext(nc) as tc:
        with (
            tc.tile_pool(name="persist", bufs=1) as pp,
            tc.tile_pool(name="sc", bufs=2, space="PSUM") as scp,
            tc.tile_pool(name="oacc", bufs=1, space="PSUM") as opp,
            tc.tile_pool(name="pj", bufs=2, space="PSUM") as pjp,
            tc.tile_pool(name="pt", bufs=6) as ptp,
            tc.tile_pool(name="rc", bufs=2) as rcp,
            tc.tile_pool(name="onm", bufs=4) as onp,
            tc.tile_pool(name="schr", bufs=2) as shp,
            tc.tile_pool(name="osb", bufs=6) as oup,
        ):
            xh_sb = pp.tile([128, KC, S], FP8, tag="xh", name="xh_sb")
            xl_sb = pp.tile([128, KC, S], FP8, tag="xl", name="xl_sb")
            w8 = pp.tile([128, 6, KC, DHC], FP8, tag="w8", name="w8_sb")
            w_sb = {nm: w8[:, i] for i, nm in enumerate(WNAMES)}
            wo_sb = pp.tile([128, 2, D], BF, tag="wo", name="wo_sb")
            # drain targets: [128 rows = head-pair dims, hi/lo, p-block*S + s]
            qt8 = pp.tile([128, 2, 2 * S], FP8, tag="qt8", name="qt8")
            kt8 = pp.tile([128, 2, 2 * S], FP8, tag="kt8", name="kt8")
            # augmented QK layouts
            #   qaug: per head h cols h*S+s, rows [q_hi(64); q_lo(64)]
            #   kaug: per (head h, ktile t) cols (2h+t)*S+s, rows [k_x; k_x]
            qaug = pp.tile([128, HPC * S], FP8, tag="qaug", name="qaug")
            kaug = pp.tile([128, 2 * HPC * S], FP8, tag="kaug", name="kaug")
            # bf16 q/k for the pair-0 windows
            qt_bf = pp.tile([128, NQW * 512], BF, tag="qt_bf", name="qt_bf")
            kt_bf = pp.tile([128, 2048], BF, tag="kt_bf", name="kt_bf")
            v_sb = pp.tile([128, NCH * VP], BF, tag="v", name="v_sb")
            onT = pp.tile([128, 2 * S], BF, tag="onT", name="onT")
            ident = pp.tile([128, 128], BF, tag="ident", name="ident")
            if with_bias:
                bias_sb = pp.tile([1, 3 * DHC], BF, tag="bias", name="bias_sb")
                ones1 = pp.tile([1, 512], BF, tag="ones1", name="ones1")

            # --- input DMAs: weights (one fp8 blob), xh chunks, xl, wo last
            nc.sync.dma_start(
                w8[:, :, :, :].rearrange("p a b c -> p (a b c)"), w8_d[:, :]
            )
            if with_bias:
                nc.sync.dma_start(bias_sb[:, :], bias_d[:, :])
                nc.vector.memset(ones1[:, :], 1.0)
            for k in range(KC):
                nc.sync.dma_start(xh_sb[:, k, :], xh_d[k * 128:(k + 1) * 128, :])
            for k in range(KC):
                nc.sync.dma_start(xl_sb[:, k, :], xl_d[k * 128:(k + 1) * 128, :])
            nc.sync.dma_start(
                wo_sb[:, :, :], wo_d[:, :].rearrange("(r p) d -> p r d", p=128)
            )

            from concourse.masks import make_identity
            make_identity(nc, ident[:, :])
            nc.gpsimd.memset(v_sb[:, :], 1.0)  # ones cols of V_aug

            # PE warmup: keep the tensor engine busy through the x-DMA
            # stream so it reaches full p-state before the lead matmuls
            warm = pjp.tile([128, 128], BF, tag="pj", name="warm")
            for _ in range(40):
                nc.tensor.transpose(warm[:, :], ident[:, :], ident[:, :])

            # --- fp8 3-term projection helpers -----------------------------
            def qk_mm(ps, hi_nm, lo_nm, p, nt, term, cp):
                """term 0: xh*wh, 1: xh*wl, 2: xl*wh (DoubleRow chunk-pair)."""
                wsb = w_sb[hi_nm if term != 1 else lo_nm]
                xsb = xh_sb if term != 2 else xl_sb
                nc.tensor.matmul(
                    ps[:, :],
                    lhsT=wsb[:, 2 * cp:2 * cp + 2, p * 128:(p + 1) * 128],
                    rhs=xsb[:, 2 * cp:2 * cp + 2, nt * 512:(nt + 1) * 512],
                    start=(term == 0 and cp == 0),
                    stop=(not with_bias and term == 2 and cp == 3),
                    perf_mode=DR,
                )

            def qk_bias(ps, boff, p):
                nc.tensor.matmul(
                    ps[:, :],
                    lhsT=bias_sb[:, boff + p * 128: boff + (p + 1) * 128],
                    rhs=ones1[0:1, :],
                    start=False,
                    stop=True,
                )

            def qk_drain(ps, dst, p, nt):
                """hi/lo fp8 split of a projected [128,512] tile."""
                cols = slice(p * S + nt * 512, p * S + (nt + 1) * 512)
                nc.vector.tensor_copy(dst[:, 0, cols], ps[:, :])
                nc.vector.scalar_tensor_tensor(
                    out=dst[:, 1, cols], in0=ps[:, :], scalar=1.0,
                    in1=dst[:, 0, cols], op0=MUL, op1=SUB,
                )

            _half = {}

            def proj_qk_a(dst, hi_nm, lo_nm, p, nt):
                ps = pjp.tile(
                    [128, 512], F32, tag="pj", name=f"pj_{hi_nm}{p}{nt}"
                )
                for term in range(2):
                    for cp in range(4):
                        qk_mm(ps, hi_nm, lo_nm, p, nt, term, cp)
                _half[(hi_nm, p, nt)] = ps

            def proj_qk_b(dst, hi_nm, lo_nm, boff, p, nt):
                """dst None -> bf16 drain into qt_bf window slot nt."""
                ps = _half.pop((hi_nm, p, nt))
                for cp in range(4):
                    qk_mm(ps, hi_nm, lo_nm, p, nt, 2, cp)
                if with_bias:
                    qk_bias(ps, boff, p)
                if dst is None:
                    nc.vector.tensor_copy(
                        qt_bf[:, nt * 512:(nt + 1) * 512], ps[:, :]
                    )
                else:
                    qk_drain(ps, dst, p, nt)

            def proj_v(sb):
                """V_aug s-block sb in natural layout [128 s, 256 d]."""
                ps = pjp.tile([128, 256], F32, tag="pj", name=f"pv_{sb}")
                for term in range(3):
                    wsb = w_sb["vh" if term != 1 else "vl"]
                    xsb = xh_sb if term != 2 else xl_sb
                    for cp in range(4):
                        nc.tensor.matmul(
                            ps[:, :],
                            lhsT=xsb[:, 2 * cp:2 * cp + 2,
                                     sb * 128:(sb + 1) * 128],
                            rhs=wsb[:, 2 * cp:2 * cp + 2, :],
                            start=(term == 0 and cp == 0),
                            stop=(not with_bias and term == 2 and cp == 3),
                            perf_mode=DR,
                        )
                if with_bias:
                    nc.tensor.matmul(
                        ps[:, :],
                        lhsT=ones1[0:1, 0:128],
                        rhs=bias_sb[:, 2 * DHC:3 * DHC],
                        start=False,
                        stop=True,
                    )
                dst3 = v_sb[:, sb * VP:(sb + 1) * VP].rearrange(
                    "p (h e) -> p h e", e=HD + 1
                )[:, :, 0:HD]
                nc.vector.tensor_copy(dst3, ps[:, :])

            # --- aug-layout remap DMAs ------------------------------------
            def remap_k(p, nt_lo, nt_hi, eng=None):
                """kaug[(h,t) cols] <- kt8: one DMA per (head, half)."""
                eng = eng or nc.sync
                c0, c1 = nt_lo * 512, nt_hi * 512
                w = c1 - c0
                for i in range(2):
                    h = 2 * p + i
                    src = kt8[64 * i:64 * (i + 1), :, p * S + c0: p * S + c1]
                    dst = kaug[:, 2 * h * S: 2 * (h + 1) * S].rearrange(
                        "p (t s) -> p t s", t=2
                    )[:, :, c0:c0 + w]
                    for half in range(2):
                        eng.dma_start(dst[64 * half:64 * half + 64, :, :], src)

            def remap_q(p, nt_lo, nt_hi, eng=None):
                """qaug rows [q_hi; q_lo] per head: one DMA per (head, t)."""
                eng = eng or nc.sync
                c0, c1 = nt_lo * 512, nt_hi * 512
                for i in range(2):
                    h = 2 * p + i
                    for t in range(2):
                        eng.dma_start(
                            qaug[64 * t:64 * t + 64, h * S + c0: h * S + c1],
                            qt8[64 * i:64 * (i + 1), t, p * S + c0: p * S + c1],
                        )

            # --- output projection (both pairs accumulated) ---------------
            def outproj(sb, on_act=False):
                ot = oup.tile([128, 1024], BF, tag="osb", name=f"ot_{sb}")
                for n in range(2):
                    po = pjp.tile([128, 512], F32, tag="pj", name=f"po_{sb}_{n}")
                    for hp in range(2):
                        nc.tensor.matmul(
                            po[:, :],
                            lhsT=onT[:, hp * S + sb * 128: hp * S + (sb + 1) * 128],
                            rhs=wo_sb[:, hp, n * 512:(n + 1) * 512],
                            start=(hp == 0),
                            stop=(hp == 1),
                        )
                    if on_act and n == 1:
                        nc.scalar.copy(ot[:, n * 512:(n + 1) * 512], po[:, :])
                    else:
                        nc.vector.tensor_copy(
                            ot[:, n * 512:(n + 1) * 512], po[:, :]
                        )
                nc.sync.dma_start(
                    out_d[sb * 128:(sb + 1) * 128, :], ot[:, :]
                )

            # --- window drain: normalize O, transpose to O^T --------------
            def drain_recip(hp, qw, oacc):
                rs = rcp.tile([128, 8], F32, tag="rs", name=f"rs_{hp}{qw}")
                nc.vector.tensor_copy(
                    rs[:, 0:7].rearrange("p (j e) -> p j e", e=1),
                    oacc[:, 0:7 * 65].rearrange(
                        "p (j e) -> p j e", e=65)[:, :, 64:65],
                )
                nc.vector.tensor_copy(
                    rs[:, 7:8], oacc[:, OFFS[7] + 64:OFFS[7] + 65]
                )
                r = rcp.tile([128, 8], F32, tag="rc", name=f"rc_{hp}{qw}")
                nc.vector.reciprocal_approx_fast(out=r[:, :], in_=rs[:, :])
                return r

            def drain_norm(hp, qw, oacc, r, qb, on_act=False):
                onm = onp.tile([128, 128], BF, tag="onm", name=f"on_{hp}{qw}{qb}")
                for i in range(2):
                    j = qb * 2 + i
                    if on_act and i == 1:
                        nc.scalar.activation(
                            onm[:, 64 * i:64 * i + 64],
                            oacc[:, OFFS[j]:OFFS[j] + 64],
                            mybir.ActivationFunctionType.Copy,
                            scale=r[:, j:j + 1],
                        )
                    else:
                        nc.vector.tensor_scalar(
                            out=onm[:, 64 * i:64 * i + 64],
                            in0=oacc[:, OFFS[j]:OFFS[j] + 64],
                            scalar1=r[:, j:j + 1],
                            scalar2=None,
                            op0=MUL,
                        )
                return onm

            def drain_tp(hp, qw, qb, onm):
                tp = pjp.tile([128, 128], BF, tag="pj", name=f"tp_{hp}{qw}{qb}")
                nc.tensor.transpose(tp[:, :], onm[:, :], ident[:, :])
                nc.vector.tensor_copy(
                    onT[:, hp * S + qw * 512 + qb * 128:
                        hp * S + qw * 512 + (qb + 1) * 128],
                    tp[:, :],
                )

            # --- lead-in: k-proj p0 nt0/nt1 + q-proj p0 nt0 (3 PSUM tiles,
            # keeping the sc ring free so QK(0,0) can issue immediately).
            # Window 0 runs fully on bf16 q/k (NBF=16); k-nt2/nt3 projections
            # run as early window-0 fillers.
            lead = [
                ("k", kt8, 0, 0, pjp, "pj"), ("k", kt8, 0, 1, pjp, "pj"),
                ("q", qt8, 0, 0, opp, "oacc"),
            ]
            lead_ps = [
                pool.tile([128, 512], F32, tag=tag, name=f"lead_{w}{p}{nt}")
                for w, dst, p, nt, pool, tag in lead
            ]
            for term in range(3):
                for cp in range(4):
                    for (w, dst, p, nt, pool, tag), ps in zip(lead, lead_ps):
                        qk_mm(ps, f"{w}h", f"{w}l", p, nt, term, cp)
            if with_bias:
                for li in range(3):
                    w, dst, p, nt, pool, tag = lead[li]
                    qk_bias(lead_ps[li], 0 if w == "q" else DHC, p)
            with tc.high_priority(10 ** 6):
                nc.vector.tensor_copy(kt_bf[:, 0:512], lead_ps[0][:, :])
                nc.scalar.copy(qt_bf[:, 0:512], lead_ps[2][:, :])
                nc.vector.tensor_copy(kt_bf[:, 512:1024], lead_ps[1][:, :])

            def proj_k_lead(nt):
                """k-proj p0 nt (2 or 3), bf16 drain only."""
                ps = pjp.tile([128, 512], F32, tag="pj", name=f"pk0{nt}")
                for term in range(3):
                    for cp in range(4):
                        qk_mm(ps, "kh", "kl", 0, nt, term, cp)
                if with_bias:
                    qk_bias(ps, DHC, 0)
                nc.vector.tensor_copy(
                    kt_bf[:, nt * 512:(nt + 1) * 512], ps[:, :]
                )

            # --- static filler schedule -----------------------------------
            fillers = {}

            def add(wi, c, fn, front=False):
                lst = fillers.setdefault((wi, c), [])
                lst.insert(0, fn) if front else lst.append(fn)

            def add_qk(wi, c, dst, hi, lo, boff, p, nt, step=1):
                # INVARIANT: at most one other pj-tag allocation may occur
                # between the _a and _b parts (pjp ring depth 2); _b goes at
                # the front of its slot so it frees the slot before any new
                # allocation in that slot.
                add(wi, c, lambda: proj_qk_a(dst, hi, lo, p, nt))
                add(wi, c + step,
                    lambda: proj_qk_b(dst, hi, lo, boff, p, nt), front=True)

            # w0: k-nt2/3 bf16 proj, v JIT, q p0 nt1 (bf16)
            add(0, 0, lambda: proj_v(0))
            add(0, 1, lambda: proj_k_lead(2))
            add(0, 2, lambda: proj_v(1))
            add(0, 3, lambda: proj_k_lead(3))
            add(0, 4, lambda: proj_v(2))
            add(0, 4, lambda: proj_v(3))
            for t in range(4, NCH):
                add(0, t, lambda t=t: proj_v(t))
            add_qk(0, 9, None, "qh", "ql", 0, 0, 1, step=2)
            # w1: k p1 fp8 (4 tiles) + remap; q p0 nt2 (bf16)
            for nt in range(NQW):
                add_qk(1, 2 * nt, kt8, "kh", "kl", DHC, 1, nt)
            add(1, 8, lambda: remap_k(1, 0, 2))
            add(1, 10, lambda: remap_k(1, 2, NQW))
            add_qk(1, 11, None, "qh", "ql", 0, 0, 2)
            # w2: q p0 nt3 (bf16); q p1 nt0, nt1 (fp8)
            add_qk(2, 0, None, "qh", "ql", 0, 0, 3)
            add_qk(2, 4, qt8, "qh", "ql", 0, 1, 0)
            add(2, 6, lambda: remap_q(1, 0, 1))
            add_qk(2, 9, qt8, "qh", "ql", 0, 1, 1)
            add(2, 11, lambda: remap_q(1, 1, 2))
            # w3: q p1 nt2, nt3 (fp8)
            add_qk(3, 0, qt8, "qh", "ql", 0, 1, 2)
            add(3, 2, lambda: remap_q(1, 2, 3))
            add_qk(3, 5, qt8, "qh", "ql", 0, 1, 3)
            add(3, 7, lambda: remap_q(1, 3, NQW))
            # outproj: sb group 4*qw..4*qw+3 ready after window 4+qw drains
            for qw in range(NQW - 1):
                for k in range(4):
                    add(5 + qw, 3 + 3 * k, lambda sb=4 * qw + k: outproj(sb))

            # --- attention windows ----------------------------------------
            pending = []
            _dbg_last = {}

            def emit_drain_stage(c, on_act=False):
                if not pending:
                    return
                hp, qw, oacc, st = pending[0]
                if debug and c == 2:
                    _dbg_last["r"] = st["r"]
                    _dbg_last["onm"] = st["onm"]
                if c == 0:
                    st["r"] = drain_recip(hp, qw, oacc)
                    st["onm"] = [drain_norm(hp, qw, oacc, st["r"], qb, on_act)
                                 for qb in range(2)]
                elif c == 1:
                    st["onm"] += [drain_norm(hp, qw, oacc, st["r"], qb, on_act)
                                  for qb in range(2, 4)]
                elif c <= 5:
                    drain_tp(hp, qw, c - 2, st["onm"][c - 2])
                    if c == 5:
                        pending.pop(0)

            LAG = 3
            scs = {}

            def emit_qk(wi, c):
                """QK for (window wi, chunk c) -> sc tile (hoisted 1 slot
                ahead of its exp so PE fillers can't delay the ACT stream)."""
                hp, qw = wi // NQW, wi % NQW
                with tc.high_priority(10 ** 6):
                    sc = scp.tile(
                        [128, 1024], F32, tag="sc", name=f"s_{hp}{qw}{c}"
                    )
                    for i in range(2):
                        h = 2 * hp + i
                        if hp == 0:
                            nc.tensor.matmul(
                                sc[:, 512 * i:512 * (i + 1)],
                                lhsT=kt_bf[64 * i:64 * (i + 1),
                                           c * 128:(c + 1) * 128],
                                rhs=qt_bf[64 * i:64 * (i + 1),
                                          qw * 512:(qw + 1) * 512],
                                start=True,
                                stop=True,
                            )
                            continue
                        nc.tensor.matmul(
                            sc[:, 512 * i:512 * (i + 1)],
                            lhsT=kaug[:, 2 * h * S:2 * (h + 1) * S].rearrange(
                                "p (t s) -> p t s", t=2
                            )[:, :, c * 128:(c + 1) * 128],
                            rhs=qaug[:, h * S + qw * 512:
                                     h * S + (qw + 1) * 512]
                            .rearrange("p (o n) -> p o n", o=1)
                            .broadcast_to([128, 2, 512]),
                            start=True,
                            stop=True,
                            perf_mode=DR,
                        )
                scs[(wi, c)] = sc

            prev_pvs = []        # deferred PV thunks from the previous window
            emit_qk(0, 0)
            for wi in range(NW):
                hp, qw = wi // NQW, wi % NQW
                oacc = opp.tile([128, 1024], F32, tag="oacc", name=f"o_{hp}{qw}")
                pts = {}

                def emit_pv(c, oacc=oacc, hp=hp, pts=pts):
                    pt_t = pts.pop(c)
                    for qb in range(4):
                        for i in range(2):
                            j = qb * 2 + i
                            # start=True would zero the whole 2KB PSUM
                            # zero-region (bank), racing the previous window's
                            # norm reads (invisible to subtile dep tracking).
                            # The tile is DVE-memset to zero instead, so every
                            # PV accumulates with start=False.
                            nc.tensor.matmul(
                                oacc[:, OFFS[j]:OFFS[j] + 65],
                                lhsT=pt_t[:, i * 512 + qb * 128:
                                          i * 512 + (qb + 1) * 128],
                                rhs=v_sb[:, c * VP + (2 * hp + i) * (HD + 1):
                                         c * VP + (2 * hp + i) * (HD + 1) + 65],
                                start=False,
                                stop=(c == NCH - 1),
                                skip_group_check=True,
                            )

                for c in range(NCH):
                    sc = scs.pop((wi, c))
                    if c + 1 < NCH:
                        emit_qk(wi, c + 1)
                    elif wi + 1 < NW:
                        emit_qk(wi + 1, 0)
                    pt_t = ptp.tile([128, 1024], BF, tag="pt", name=f"p_{hp}{qw}{c}")
                    with tc.high_priority(10 ** 6):
                        if (wi, c) in SCHR:
                            it = shp.tile(
                                [128, 1024], I32, tag="si", name=f"i_{hp}{qw}{c}"
                            )
                            nc.vector.tensor_scalar(
                                out=it[:, :], in0=sc[:, :],
                                scalar1=SCHR_A, scalar2=SCHR_B,
                                op0=MUL, op1=ADD,
                            )
                            nc.gpsimd.tensor_copy(
                                pt_t[:, :], it[:, :].bitcast(F32)
                            )
                        else:
                            nc.scalar.activation(
                                pt_t[:, :], sc[:, :], EXP, scale=EXPSC
                            )
                    pts[c] = pt_t

                    if c == 0:
                        for f in prev_pvs:   # all trailing PVs BEFORE the
                            f()              # drain reads the old oacc
                        prev_pvs = []
                    for fn in fillers.get((wi, c), ()):
                        fn()
                    emit_drain_stage(c)
                    if c == 2:
                        nc.vector.memset(oacc[:, :], 0.0)
                    if c >= LAG:
                        emit_pv(c - LAG)

                prev_pvs = [
                    (lambda c=c, f=emit_pv: f(c))
                    for c in range(NCH - LAG, NCH)
                ]
                pending.append((hp, qw, oacc, {}))

            # --- tail: trailing PVs, last drain, last outproj group --------
            for f in prev_pvs:
                f()
            if debug:
                nc.sync.dma_start(dbg["v"][:, :], v_sb[:, :])
                nc.sync.dma_start(dbg["ktbf"][:, :], kt_bf[:, :])
                nc.sync.dma_start(dbg["qtbf"][:, :], qt_bf[:, :])
                nc.sync.dma_start(dbg["qaug"][:, :], qaug[:, :])
                nc.sync.dma_start(dbg["kaug"][:, :], kaug[:, :])
                _, _, oacc_last, _ = pending[0]
                dbg_o = pp.tile([128, 1024], F32, tag="dbgo", name="dbg_o")
                nc.vector.tensor_copy(dbg_o[:, :], oacc_last[:, :])
                nc.sync.dma_start(dbg["oacc"][:, :], dbg_o[:, :])
            for c in range(3):
                emit_drain_stage(c, on_act=True)
            sbs = list(range(4 * (NQW - 1), NCH))
            for c in range(3, 6):
                emit_drain_stage(c)
                outproj(sbs[c - 3], on_act=True)
            outproj(sbs[3], on_act=True)
            if debug:
                nc.sync.dma_start(dbg["onT"][:, :], onT[:, :])
                nc.sync.dma_start(dbg["r"][:, :], _dbg_last["r"][:, :])
                for qb in range(4):
                    nc.sync.dma_start(
                        dbg["onm"][:, qb * 128:(qb + 1) * 128],
                        _dbg_last["onm"][qb][:, :],
                    )

    nc.compile()
    return nc


def _get_nc(with_bias=False):
    if with_bias not in _nc_cache:
        _nc_cache[with_bias] = _build_bass(with_bias=with_bias)
    return _nc_cache[with_bias]


def _prepare_in_maps(x, wq, bq, wk, bk, wv, bv, wo, with_bias):
    import ml_dtypes

    f8 = ml_dtypes.float8_e4m3
    bf16 = ml_dtypes.bfloat16
    x = np.asarray(x, np.float32)
    wq, bq = np.asarray(wq, np.float32), np.asarray(bq, np.float32)
    wk, bk = np.asarray(wk, np.float32), np.asarray(bk, np.float32)
    wv, bv = np.asarray(wv, np.float32), np.asarray(bv, np.float32)
    wo = np.asarray(wo, np.float32)

    def split8(a):
        hi = a.astype(f8)
        lo = (a - hi.astype(np.float32)).astype(f8)
        return hi, lo

    xh, xl = [], []
    for b in range(B):
        h, l = split8(x[b].T)
        xh.append(np.ascontiguousarray(h))
        xl.append(np.ascontiguousarray(l))

    def wchunks(w):
        # [D, DHC] -> [128, KC*DHC] (chunk-major columns)
        return w.reshape(KC, 128, DHC).transpose(1, 0, 2).reshape(128, KC * DHC)

    in_maps = []
    for c in range(NC):
        b, j = divmod(c, HPC)
        cs = slice(DHC * j, DHC * (j + 1))
        parts = []
        for w in (wq[:, cs], wk[:, cs], wv[:, cs]):
            h, l = split8(w * WSC)
            parts += [wchunks(h), wchunks(l)]
        # order: qh, ql, kh, kl, vh, vl
        w8 = np.concatenate(parts, axis=1)
        m = {
            "xh": xh[b],
            "xl": xl[b],
            "w8": np.ascontiguousarray(w8),
            "wo_c": np.ascontiguousarray(wo[cs, :] / WSC).astype(bf16),
        }
        if with_bias:
            bias3 = np.concatenate(
                [bq[cs], bk[cs], bv[cs]]).reshape(1, 3 * DHC) * WSC
            m["bias3"] = np.ascontiguousarray(bias3.astype(bf16))
        in_maps.append(m)
    return in_maps


def kernel(x, wq, bq, wk, bk, wv, bv, wo, bo):
    from concourse import bass_utils

    with_bias = bool(
        np.any(np.asarray(bq)) or np.any(np.asarray(bk)) or np.any(np.asarray(bv))
    )
    in_maps = _prepare_in_maps(x, wq, bq, wk, bk, wv, bv, wo, with_bias)
    res = bass_utils.run_bass_kernel_spmd(
        nc=_get_nc(with_bias), in_maps=in_maps, core_ids=list(range(NC))
    )
    bo = np.asarray(bo, np.float32)
    out = np.empty((B, S, D), np.float32)
    for b in range(B):
        acc = np.asarray(res.results[HPC * b]["out"], np.float32)
        for j in range(1, HPC):
            acc = acc + np.asarray(res.results[HPC * b + j]["out"], np.float32)
        out[b] = acc + bo
    return out
